# revision 24
# baseline (speedup 1.0000x reference)
"""Trainium2 Bass kernel for nn_AttentionSeqToMasked (dense transformer attention).

Full-input contract: kernel(**inputs) takes the unsharded numpy inputs and
returns the full [B, SQ, H*D_V] float32 output.

Sharding (8 cores): data parallel over batch (B=4 -> 2 cores per batch) x
tensor parallel over heads (16 heads -> 8 per core). Each core computes
attention for one (batch, head-half) pair; host gathers the slices.

Per-core dataflow (all matmuls bf16 inputs, fp32 PSUM accumulation):
  - Host pre-transposes activations to X^T [D_PRE, S] bf16 so the contraction
    dim (D_PRE) lands on SBUF partitions with fully-contiguous DMA loads.
  - Projections compute q^T/k^T = W^T @ X^T directly (head-dim on partitions),
    v in natural [s, d_v] layout with a ones-column appended via the weight
    matrix (zero weight column + bias 1.0).
  - Scores are computed transposed: scoresT[k, q] = kT.T @ qT, two heads
    packed into the 128x128 PE array per matmul pair (d_head=64 row groups).
  - Key-mask folds into the exp as a per-partition bias (0 or -30000);
    1/sqrt(d) folds into the exp scale. No max-subtraction is needed
    (logits are O(1) by construction; exp cannot overflow fp32).
  - AV matmul contracts exp(scores)T with [v | ones]: row 64 of the psum is
    the softmax denominator, computed for free alongside the numerator.
  - A final PE transpose returns [q, d_v+1] tiles; VectorE multiplies by the
    reciprocal denominator and the result DMAs straight to DRAM.

Scheduling: projection work for pair p+1 is chopped into ~1.7us psum-chunks
and interleaved into pair p's attention loop every 4 key-tiles, keeping the
TensorE fed while ScalarE (exp) is the steady-state bottleneck.
"""

import os
from contextlib import ExitStack

import numpy as np
import ml_dtypes

import concourse.bass as bass
import concourse.bacc as bacc
import concourse.mybir as mybir
import concourse.tile as tile
from concourse.bass_utils import run_bass_kernel_spmd
from concourse.masks import make_identity

# Problem shape (hardcoded per contract)
B, SQ, SK = 4, 2048, 2048
D_PRE = 1024
H, D_QK, D_V = 16, 64, 64
N_CORES = 8
HALF = (H // 2) * D_QK  # 512 columns of the projection handled per core
N_PAIRS = 4  # head pairs per core
S_CHUNK = 512  # moving free-dim per matmul
N_DT = D_PRE // 128  # d_pre tiles of 128
N_KT = SK // 128  # key tiles of 128
N_QC = SQ // S_CHUNK  # query chunks of 512
MASK_NEG = -30000.0

F32 = mybir.dt.float32
BF16 = mybir.dt.bfloat16
BF16_NP = np.dtype(ml_dtypes.bfloat16)

_COMPILED = None


def _build_program():
    nc = bacc.Bacc("TRN2", target_bir_lowering=False, debug=False)

    # DRAM I/O (names are the in_map keys)
    xq = nc.dram_tensor("xq", [N_DT, 128, SQ], BF16, kind="ExternalInput").ap()
    xk = nc.dram_tensor("xk", [N_DT, 128, SK], BF16, kind="ExternalInput").ap()
    xv = nc.dram_tensor("xv", [N_DT, 128, SK], BF16, kind="ExternalInput").ap()
    wq = nc.dram_tensor("wq", [N_DT, 128, HALF], BF16, kind="ExternalInput").ap()
    wk = nc.dram_tensor("wk", [N_DT, 128, HALF], BF16, kind="ExternalInput").ap()
    # v weights with a zero column appended per head (ones column generator)
    wv = nc.dram_tensor("wv", [N_DT, 128, N_PAIRS * 130], BF16, kind="ExternalInput").ap()
    bq = nc.dram_tensor("bq", [128, N_PAIRS], F32, kind="ExternalInput").ap()
    bk = nc.dram_tensor("bk", [128, N_PAIRS], F32, kind="ExternalInput").ap()
    bv = nc.dram_tensor("bv", [128, N_PAIRS * 130], F32, kind="ExternalInput").ap()
    mb = nc.dram_tensor("mb", [128, N_KT], F32, kind="ExternalInput").ap()
    out = nc.dram_tensor("out", [SQ // 128, 128, HALF], F32, kind="ExternalOutput").ap()

    with tile.TileContext(nc) as tc:
        _emit(tc, xq, xk, xv, wq, wk, wv, bq, bk, bv, mb, out)

    nc.compile()
    return nc


def _emit(tc, xq, xk, xv, wq, wk, wv, bq, bk, bv, mb, out):
    nc = tc.nc

    with ExitStack() as ctx:
        # ---- pools ----
        xp = ctx.enter_context(tc.tile_pool(name="x", bufs=3))
        wp = ctx.enter_context(tc.tile_pool(name="w", bufs=1))
        cp = ctx.enter_context(tc.tile_pool(name="const", bufs=1))
        qkvp = ctx.enter_context(tc.tile_pool(name="qkv", bufs=1))
        expp = ctx.enter_context(tc.tile_pool(name="exp", bufs=3))
        avtp = ctx.enter_context(tc.tile_pool(name="avt", bufs=2))
        stgp = ctx.enter_context(tc.tile_pool(name="stg", bufs=3))
        rp = ctx.enter_context(tc.tile_pool(name="recip", bufs=8))

        proj_ps = ctx.enter_context(tc.tile_pool(name="proj_ps", bufs=1, space="PSUM"))
        sc_ps = ctx.enter_context(tc.tile_pool(name="sc_ps", bufs=2, space="PSUM"))
        av_ps = ctx.enter_context(tc.tile_pool(name="av_ps", bufs=2, space="PSUM"))
        tp_ps = ctx.enter_context(tc.tile_pool(name="tp_ps", bufs=1, space="PSUM"))

        # ---- constants ----
        ident = cp.tile([128, 128], F32, name="ident")
        make_identity(nc, ident)
        mb_sb = cp.tile([128, N_KT], F32, name="mb_sb")
        nc.sync.dma_start(mb_sb, mb)
        bq_sb = cp.tile([128, N_PAIRS], F32, name="bq_sb")
        nc.sync.dma_start(bq_sb, bq)
        bk_sb = cp.tile([128, N_PAIRS], F32, name="bk_sb")
        nc.sync.dma_start(bk_sb, bk)
        bv_sb = cp.tile([128, N_PAIRS * 130], F32, name="bv_sb")
        nc.sync.dma_start(bv_sb, bv)

        # ---- streamed loads as one 3D "mega" tile per tensor: each DMA
        # instruction covers one 512-column chunk across all 8 dt tiles, so
        # the Sync engine issues 15 input DMAs instead of ~120 (issue cost is
        # ~0.6us per DMA instruction on the queue-owning engine). ----
        def alloc_x(pfx):
            mega = xp.tile([128, N_DT, SQ], BF16, name=f"{pfx}m", tag="x")
            return mega, [mega[:, dt_i, :] for dt_i in range(N_DT)]

        def load_x_chunk(mega, xap, c):
            lo, hi = c * S_CHUNK, (c + 1) * S_CHUNK
            nc.sync.dma_start(
                mega[:, :, lo:hi], xap[:, :, lo:hi].rearrange("d p c -> p d c")
            )

        def alloc_w(pfx, width):
            mega = wp.tile([128, N_DT, width], BF16, name=f"{pfx}m", tag=f"{pfx}m")
            return mega, [mega[:, dt_i, :] for dt_i in range(N_DT)]

        def load_w_cols(mega, wap, lo, hi):
            nc.sync.dma_start(
                mega[:, :, lo:hi], wap[:, :, lo:hi].rearrange("d p c -> p d c")
            )

        xq_m, xq_sb = alloc_x("xq")
        xk_m, xk_sb = alloc_x("xk")
        xv_m, xv_sb = alloc_x("xv")
        wq_m, wq_sb = alloc_w("wq", HALF)
        wk_m, wk_sb = alloc_w("wk", HALF)
        wv_m, wv_sb = alloc_w("wv", N_PAIRS * 130)
        # arrival order == first-use order of the filler/attention streams;
        # weights are pair/group-sliced so the first scores wait on ~2.5MB
        load_w_cols(wq_m, wq, 0, 128)
        load_x_chunk(xq_m, xq, 0)
        load_w_cols(wk_m, wk, 0, 128)
        load_x_chunk(xk_m, xk, 0)
        load_x_chunk(xk_m, xk, 1)
        load_x_chunk(xk_m, xk, 2)
        load_x_chunk(xk_m, xk, 3)
        load_w_cols(wv_m, wv, 0, 260)
        load_x_chunk(xv_m, xv, 0)
        load_x_chunk(xq_m, xq, 1)
        load_w_cols(wq_m, wq, 128, HALF)
        load_w_cols(wk_m, wk, 128, HALF)
        load_x_chunk(xv_m, xv, 1)
        load_w_cols(wv_m, wv, 260, N_PAIRS * 130)
        load_x_chunk(xv_m, xv, 2)
        load_x_chunk(xq_m, xq, 2)
        load_x_chunk(xv_m, xv, 3)
        load_x_chunk(xq_m, xq, 3)

        v_tiles = {}  # (pair, kt) -> [128, 130] bf16 tile
        qkT = {}  # (pfx, pair) -> [128, SQ] bf16 tile
        qk_done = set()  # (pfx, pair, qc) fully emitted projection chunks

        def qk_tile(pfx, pair):
            if (pfx, pair) not in qkT:
                qkT[(pfx, pair)] = qkvp.tile(
                    [128, SQ], BF16, name=f"{pfx}T{pair}", tag=f"{pfx}T", bufs=2
                )
            return qkT[(pfx, pair)]

        proj_ps_open = {}

        def emit_qk_chunk(pair, pfx, qc, half=None):
            # one [128, 512] projection chunk: 8 accumulating MMs + bias copy.
            # half=0/1 emits only the first/second 4 contraction MMs (filler
            # granularity); half=None emits the whole chunk.
            dst = qk_tile(pfx, pair)
            w_sb = wq_sb if pfx == "q" else wk_sb
            b_sb = bq_sb if pfx == "q" else bk_sb
            x_sb = xq_sb if pfx == "q" else xk_sb
            key = (pair, pfx, qc)
            if half == 1:
                ps = proj_ps_open.pop(key)
            else:
                ps = proj_ps.tile(
                    [128, S_CHUNK], F32, name=f"{pfx}ps{pair}_{qc}", tag="proj"
                )
            dts = range(N_DT) if half is None else range(half * 4, half * 4 + 4)
            for dt_i in dts:
                nc.tensor.matmul(
                    ps,
                    lhsT=w_sb[dt_i][:, pair * 128 : (pair + 1) * 128],
                    rhs=x_sb[dt_i][:, qc * S_CHUNK : (qc + 1) * S_CHUNK],
                    start=(dt_i == 0),
                    stop=(dt_i == N_DT - 1),
                )
            if half == 0:
                proj_ps_open[key] = ps
            else:
                nc.vector.tensor_scalar_add(
                    dst[:, qc * S_CHUNK : (qc + 1) * S_CHUNK],
                    ps,
                    b_sb[:, pair : pair + 1],
                )
                qk_done.add((pfx, pair, qc))

        v_ps_open = {}

        def emit_v_chunk(g, st, half=None):
            # v projection for pairs (2g, 2g+1), one key tile: N=260 matmuls.
            # half=0/1 splits the 8 contraction MMs for filler granularity.
            if half == 1:
                ps = v_ps_open.pop((g, st))
            else:
                ps = proj_ps.tile([128, S_CHUNK], F32, name=f"vps{g}_{st}", tag="proj")
            dts = range(N_DT) if half is None else range(half * 4, half * 4 + 4)
            for dt_i in dts:
                nc.tensor.matmul(
                    ps[:, 0:260],
                    lhsT=xv_sb[dt_i][:, st * 128 : (st + 1) * 128],
                    rhs=wv_sb[dt_i][:, g * 260 : (g + 1) * 260],
                    start=(dt_i == 0),
                    stop=(dt_i == N_DT - 1),
                )
            if half == 0:
                v_ps_open[(g, st)] = ps
                return
            for j in range(2):
                pair = 2 * g + j
                vt = qkvp.tile(
                    [128, 130], BF16, name=f"v{pair}_{st}", tag="v", bufs=4 * N_KT
                )
                nc.vector.tensor_add(
                    vt,
                    ps[:, j * 130 : (j + 1) * 130],
                    bv_sb[:, pair * 130 : (pair + 1) * 130],
                )
                v_tiles[(pair, st)] = vt

        # filler queue: ALL deferred projection work in ~0.43us halves, popped
        # two per block (straddling the scores so the in-order PE queue never
        # stalls on the ACTIVATE ping-pong). Ordered by DMA arrival and by
        # need-by (python emission must precede readers).
        filler = []  # (cost_us, deadline_block, emit_fn)

        def _qk_half(pair, pfx, qc, half):
            return lambda: emit_qk_chunk(pair, pfx, qc, half)

        def _v_half(g, st, half):
            return lambda: emit_v_chunk(g, st, half)

        def add_qk(pair, pfx, qc, dl):
            filler.append((0.88, dl, _qk_half(pair, pfx, qc, 0)))
            filler.append((0.88, dl, _qk_half(pair, pfx, qc, 1)))

        def add_v(g, st, dl):
            filler.append((0.45, dl, _v_half(g, st, 0)))
            filler.append((0.45, dl, _v_half(g, st, 1)))

        # deadline = block whose emitted scores/AVs read the produced tile
        add_qk(0, "q", 1, 7)
        for st in range(0, 6):
            add_v(0, st, 8 + st)
        add_qk(0, "q", 2, 15)
        for st in range(6, N_KT):
            add_v(0, st, 8 + st)
        add_qk(0, "q", 3, 23)
        for c in range(N_QC):
            add_qk(1, "k", c, 31 + 2 * c)
        add_qk(1, "q", 0, 31)
        for st in range(N_KT):
            add_v(1, st, 40 + st)
        add_qk(1, "q", 1, 39)
        add_qk(1, "q", 2, 47)
        add_qk(1, "q", 3, 55)
        for c in range(N_QC):
            add_qk(2, "k", c, 56 + 2 * c)
        for c in range(N_QC):
            add_qk(2, "q", c, 55 + 6 * c)
        for c in range(N_QC):
            add_qk(3, "k", c, 74 + 2 * c)
        for c in range(N_QC):
            add_qk(3, "q", c, 78 + 5 * c)

        filler.sort(key=lambda e: e[1])
        filler_total = sum(e[0] for e in filler)
        spent = [0.0]

        def pop_filler(b):
            # forced pops: deadline due next block (correctness)
            popped = 0.0
            while filler and filler[0][1] <= b + 1:
                cost, _, fn = filler.pop(0)
                fn()
                spent[0] += cost
                popped += cost
            # uniform reserve: keep global pace so the tail stays fed
            target = filler_total * (b + 1) / 105.0
            while (
                filler
                and popped < 1.0
                and spent[0] + filler[0][0] <= target + 0.9
            ):
                cost, _, fn = filler.pop(0)
                fn()
                spent[0] += cost
                popped += cost

        # prologue: first-scores deps, plus the k chunks (their DMA chunks
        # arrive during the otherwise PE-idle startup window)
        emit_qk_chunk(0, "q", 0)
        emit_qk_chunk(0, "k", 0)
        for c in range(1, N_QC):
            emit_qk_chunk(0, "k", c)

        # ---- software-pipelined attention stream over (pair, qc, kt) ----
        iters = [
            (pair, qc, kt)
            for pair in range(N_PAIRS)
            for qc in range(N_QC)
            for kt in range(N_KT)
        ]
        sc_map = {}
        av_map = {}

        def emit_scores(i):
            pair, qc, kt = iters[i]
            assert ("q", pair, qc) in qk_done, f"q chunk not ready for iter {i}"
            assert ("k", pair, kt // 4) in qk_done, f"k chunk not ready for iter {i}"
            qT = qk_tile("q", pair)
            kT = qk_tile("k", pair)
            sc = sc_ps.tile([128, 1024], F32, name=f"sc{pair}_{qc}_{kt}", tag="sc")
            # scoresT for heads A and B, packed in PE row groups
            nc.tensor.matmul(
                sc[:, 0:512],
                lhsT=kT[0:64, kt * 128 : (kt + 1) * 128],
                rhs=qT[0:64, qc * S_CHUNK : (qc + 1) * S_CHUNK],
                start=True,
                stop=True,
            )
            nc.tensor.matmul(
                sc[:, 512:1024],
                lhsT=kT[64:128, kt * 128 : (kt + 1) * 128],
                rhs=qT[64:128, qc * S_CHUNK : (qc + 1) * S_CHUNK],
                start=True,
                stop=True,
            )
            sc_map[i] = sc

        def emit_epilogue(pair, qc, av_a, av_b):
            # transpose back to [q, d_v], normalize, store
            stg3 = stgp.tile([128, 4, 128], F32, name=f"st{pair}_{qc}", tag="stg")
            stgs = [stg3[:, u, :] for u in range(4)]
            for h_i, av in enumerate((av_a, av_b)):
                avt = avtp.tile(
                    [65, S_CHUNK], F32, name=f"avt{pair}_{qc}_{h_i}", tag="avt"
                )
                nc.vector.tensor_copy(avt, av)
                tp = tp_ps.tile([128, 260], F32, name=f"tp{pair}_{qc}_{h_i}", tag="tp")
                for u in range(4):
                    nc.tensor.transpose(
                        tp[:, u * 65 : u * 65 + 65],
                        avt[:, u * 128 : (u + 1) * 128],
                        ident[0:65, 0:65],
                    )
                rc = rp.tile([128, 4], F32, name=f"rc{pair}_{qc}_{h_i}", tag="rc")
                nc.vector.reciprocal(rc, tp[:, 64:260:65])
                for u in range(4):
                    nc.vector.tensor_scalar_mul(
                        stgs[u][:, h_i * 64 : (h_i + 1) * 64],
                        tp[:, u * 65 : u * 65 + 64],
                        rc[:, u : u + 1],
                    )
            nc.sync.dma_start(
                out[qc * 4 : (qc + 1) * 4, :, pair * 128 : (pair + 1) * 128]
                .rearrange("u p c -> p u c"),
                stg3,
            )

        def emit_av(pair, qc, kt, ex):
            if kt == 0:
                av_map[(pair, qc)] = (
                    av_ps.tile([65, S_CHUNK], F32, name=f"ava{pair}_{qc}", tag="av"),
                    av_ps.tile([65, S_CHUNK], F32, name=f"avb{pair}_{qc}", tag="av"),
                )
            av_a, av_b = av_map[(pair, qc)]
            nc.tensor.matmul(
                av_a,
                lhsT=v_tiles[(pair, kt)][:, 0:65],
                rhs=ex[:, 0:512],
                start=(kt == 0),
                stop=(kt == N_KT - 1),
            )
            nc.tensor.matmul(
                av_b,
                lhsT=v_tiles[(pair, kt)][:, 65:130],
                rhs=ex[:, 512:1024],
                start=(kt == 0),
                stop=(kt == N_KT - 1),
            )

        # Emission in 2-iteration blocks, software-pipelined:
        #   block b: exps (2b, 2b+1) | AV burst (2b-2, 2b-1) | scores (2b+2,
        #   2b+3) | one filler unit. The AV inputs are always two blocks old,
        #   so the 4-matmul AV burst never waits mid-stream; batching halves
        #   the PE stream-switch tax. Iters 0..15 (pair 0, qc 0) defer their
        #   AVs entirely so ScalarE starts while the v projection still waits
        #   on the xv DMA (~50us).
        emit_scores(0)
        emit_scores(1)
        ex_map = {}
        n_it = len(iters)

        def emit_av_i(i):
            pair, qc, kt = iters[i]
            emit_av(pair, qc, kt, ex_map.pop(i))
            if kt == N_KT - 1:
                emit_epilogue(pair, qc, *av_map.pop((pair, qc)))

        av_cursor = 0

        def av_ready(i):
            pair, qc, kt = iters[i]
            return (pair, kt) in v_tiles

        # Block b: exps (2b, 2b+1) | AV catch-up (lag-1 target, max 4, gated
        # on the v projection) | filler half | scores (2b+2, 2b+3) | filler
        # half. Fillers straddle the scores so the in-order PE queue reaches
        # the scores only after the previous ACTIVATE has freed their psum.
        for b in range(n_it // 2):
            i0, i1 = 2 * b, 2 * b + 1
            for i in (i0, i1):
                pair, qc, kt = iters[i]
                ex = expp.tile(
                    [128, 1024], BF16, name=f"ex{pair}_{qc}_{kt}", tag="ex", bufs=20
                )
                ex_map[i] = ex
                nc.scalar.activation(
                    ex,
                    sc_map.pop(i),
                    mybir.ActivationFunctionType.Exp,
                    bias=mb_sb[:, kt : kt + 1],
                    scale=0.125,
                )
            if b > 8:
                navs = 0
                while (
                    av_cursor <= min(2 * b - 1, n_it - 1)
                    and navs < 4
                    and av_ready(av_cursor)
                ):
                    emit_av_i(av_cursor)
                    av_cursor += 1
                    navs += 1
            if i1 + 2 < n_it:
                emit_scores(i1 + 1)
                emit_scores(i1 + 2)
            pop_filler(b)
        # drain remaining AVs + epilogues
        while filler:
            cost, _, fn = filler.pop(0)
            fn()
        while av_cursor < n_it:
            assert av_ready(av_cursor)
            emit_av_i(av_cursor)
            av_cursor += 1

        assert not filler, f"{len(filler)} filler chunks left unscheduled"
        assert not ex_map and not av_map and not sc_map

        assert not filler, f"{len(filler)} filler chunks left unscheduled"


def _prep_core_inputs(pre_qs, pre_ks, pre_vs, k_mask, q_w, q_b, k_w, k_b, v_w, v_b, core):
    b = core // 2
    hh = core % 2
    cols = slice(HALF * hh, HALF * (hh + 1))

    xq = np.ascontiguousarray(pre_qs[b].T).astype(BF16_NP).reshape(N_DT, 128, SQ)
    xk = np.ascontiguousarray(pre_ks[b].T).astype(BF16_NP).reshape(N_DT, 128, SK)
    xv = np.ascontiguousarray(pre_vs[b].T).astype(BF16_NP).reshape(N_DT, 128, SK)
    wq = np.ascontiguousarray(q_w[:, cols]).astype(BF16_NP).reshape(N_DT, 128, HALF)
    wk = np.ascontiguousarray(k_w[:, cols]).astype(BF16_NP).reshape(N_DT, 128, HALF)

    wv_core = v_w[:, cols].astype(np.float32)
    wv = np.zeros((D_PRE, N_PAIRS * 130), dtype=np.float32)
    bv_core = v_b[cols].astype(np.float32)
    bv_ext = np.zeros(N_PAIRS * 130, dtype=np.float32)
    for p in range(N_PAIRS):
        wv[:, p * 130 : p * 130 + 64] = wv_core[:, p * 128 : p * 128 + 64]
        wv[:, p * 130 + 65 : p * 130 + 129] = wv_core[:, p * 128 + 64 : p * 128 + 128]
        bv_ext[p * 130 : p * 130 + 64] = bv_core[p * 128 : p * 128 + 64]
        bv_ext[p * 130 + 64] = 1.0
        bv_ext[p * 130 + 65 : p * 130 + 129] = bv_core[p * 128 + 64 : p * 128 + 128]
        bv_ext[p * 130 + 129] = 1.0

    bq = np.ascontiguousarray(q_b[cols].astype(np.float32).reshape(N_PAIRS, 128).T)
    bk = np.ascontiguousarray(k_b[cols].astype(np.float32).reshape(N_PAIRS, 128).T)
    bv_full = np.ascontiguousarray(np.tile(bv_ext[None, :], (128, 1)))

    # mask True -> 0.0, False -> MASK_NEG
    mbias = np.where(k_mask[b], 0.0, MASK_NEG).astype(np.float32)
    mb = np.ascontiguousarray(mbias.reshape(N_KT, 128).T)

    return {
        "xq": xq,
        "xk": xk,
        "xv": xv,
        "wq": wq,
        "wk": wk,
        "wv": wv.astype(BF16_NP).reshape(N_DT, 128, N_PAIRS * 130),
        "bq": bq,
        "bk": bk,
        "bv": bv_full,
        "mb": mb,
    }


def kernel(pre_qs, pre_ks, pre_vs, k_mask, q_w, q_b, k_w, k_b, v_w, v_b):
    global _COMPILED
    args = (pre_qs, pre_ks, pre_vs, k_mask, q_w, q_b, k_w, k_b, v_w, v_b)
    args = tuple(np.asarray(a) for a in args)

    if _COMPILED is None:
        _COMPILED = _build_program()
    nc = _COMPILED

    in_maps = [_prep_core_inputs(*args, core=c) for c in range(N_CORES)]

    trace = bool(int(os.environ.get("BASS_KERNEL_TRACE", "0")))
    res = run_bass_kernel_spmd(
        nc,
        in_maps,
        core_ids=list(range(N_CORES)),
        trace=trace,
    )
    if trace:
        kernel.last_results = res

    out = np.empty((B, SQ, H * D_V), dtype=np.float32)
    for c in range(N_CORES):
        b = c // 2
        hh = c % 2
        out[b, :, HALF * hh : HALF * (hh + 1)] = res.results[c]["out"].reshape(SQ, HALF)
    return out



# revision 26
# speedup vs baseline: 1.0054x; 1.0054x over previous
"""Trainium2 Bass kernel for nn_AttentionSeqToMasked (dense transformer attention).

Full-input contract: kernel(**inputs) takes the unsharded numpy inputs and
returns the full [B, SQ, H*D_V] float32 output.

Sharding (8 cores): data parallel over batch (B=4 -> 2 cores per batch) x
tensor parallel over heads (16 heads -> 8 per core). Each core computes
attention for one (batch, head-half) pair; host gathers the slices.

Per-core dataflow (all matmuls bf16 inputs, fp32 PSUM accumulation):
  - Host pre-transposes activations to X^T [D_PRE, S] bf16 so the contraction
    dim (D_PRE) lands on SBUF partitions with fully-contiguous DMA loads.
  - Projections compute q^T/k^T = W^T @ X^T directly (head-dim on partitions),
    v in natural [s, d_v] layout with a ones-column appended via the weight
    matrix (zero weight column + bias 1.0).
  - Scores are computed transposed: scoresT[k, q] = kT.T @ qT, two heads
    packed into the 128x128 PE array per matmul pair (d_head=64 row groups).
  - Key-mask folds into the exp as a per-partition bias (0 or -30000);
    1/sqrt(d) folds into the exp scale. No max-subtraction is needed
    (logits are O(1) by construction; exp cannot overflow fp32).
  - AV matmul contracts exp(scores)T with [v | ones]: row 64 of the psum is
    the softmax denominator, computed for free alongside the numerator.
  - A final PE transpose returns [q, d_v+1] tiles; VectorE multiplies by the
    reciprocal denominator and the result DMAs straight to DRAM.

Scheduling: projection work for pair p+1 is chopped into ~1.7us psum-chunks
and interleaved into pair p's attention loop every 4 key-tiles, keeping the
TensorE fed while ScalarE (exp) is the steady-state bottleneck.
"""

import os
from contextlib import ExitStack

import numpy as np
import ml_dtypes

import concourse.bass as bass
import concourse.bacc as bacc
import concourse.mybir as mybir
import concourse.tile as tile
from concourse.bass_utils import run_bass_kernel_spmd
from concourse.masks import make_identity

# Problem shape (hardcoded per contract)
B, SQ, SK = 4, 2048, 2048
D_PRE = 1024
H, D_QK, D_V = 16, 64, 64
N_CORES = 8
HALF = (H // 2) * D_QK  # 512 columns of the projection handled per core
N_PAIRS = 4  # head pairs per core
S_CHUNK = 512  # moving free-dim per matmul
N_DT = D_PRE // 128  # d_pre tiles of 128
N_KT = SK // 128  # key tiles of 128
N_QC = SQ // S_CHUNK  # query chunks of 512
MASK_NEG = -30000.0

F32 = mybir.dt.float32
BF16 = mybir.dt.bfloat16
BF16_NP = np.dtype(ml_dtypes.bfloat16)

_COMPILED = None


def _build_program():
    nc = bacc.Bacc("TRN2", target_bir_lowering=False, debug=False)

    # DRAM I/O (names are the in_map keys)
    xq = nc.dram_tensor("xq", [N_DT, 128, SQ], BF16, kind="ExternalInput").ap()
    xk = nc.dram_tensor("xk", [N_DT, 128, SK], BF16, kind="ExternalInput").ap()
    xv = nc.dram_tensor("xv", [N_DT, 128, SK], BF16, kind="ExternalInput").ap()
    wq = nc.dram_tensor("wq", [N_DT, 128, HALF], BF16, kind="ExternalInput").ap()
    wk = nc.dram_tensor("wk", [N_DT, 128, HALF], BF16, kind="ExternalInput").ap()
    # v weights with a zero column appended per head (ones column generator)
    wv = nc.dram_tensor("wv", [N_DT, 128, N_PAIRS * 130], BF16, kind="ExternalInput").ap()
    bq = nc.dram_tensor("bq", [128, N_PAIRS], F32, kind="ExternalInput").ap()
    bk = nc.dram_tensor("bk", [128, N_PAIRS], F32, kind="ExternalInput").ap()
    bv = nc.dram_tensor("bv", [128, N_PAIRS * 130], F32, kind="ExternalInput").ap()
    mb = nc.dram_tensor("mb", [128, N_KT], F32, kind="ExternalInput").ap()
    out = nc.dram_tensor("out", [SQ // 128, 128, HALF], F32, kind="ExternalOutput").ap()

    with tile.TileContext(nc) as tc:
        _emit(tc, xq, xk, xv, wq, wk, wv, bq, bk, bv, mb, out)

    nc.compile()
    return nc


def _emit(tc, xq, xk, xv, wq, wk, wv, bq, bk, bv, mb, out):
    nc = tc.nc

    with ExitStack() as ctx:
        # ---- pools ----
        xp = ctx.enter_context(tc.tile_pool(name="x", bufs=3))
        wp = ctx.enter_context(tc.tile_pool(name="w", bufs=1))
        cp = ctx.enter_context(tc.tile_pool(name="const", bufs=1))
        qkvp = ctx.enter_context(tc.tile_pool(name="qkv", bufs=1))
        expp = ctx.enter_context(tc.tile_pool(name="exp", bufs=3))
        avtp = ctx.enter_context(tc.tile_pool(name="avt", bufs=2))
        stgp = ctx.enter_context(tc.tile_pool(name="stg", bufs=2))
        rp = ctx.enter_context(tc.tile_pool(name="recip", bufs=4))

        proj_ps = ctx.enter_context(tc.tile_pool(name="proj_ps", bufs=1, space="PSUM"))
        sc_ps = ctx.enter_context(tc.tile_pool(name="sc_ps", bufs=2, space="PSUM"))
        av_ps = ctx.enter_context(tc.tile_pool(name="av_ps", bufs=2, space="PSUM"))
        tp_ps = ctx.enter_context(tc.tile_pool(name="tp_ps", bufs=1, space="PSUM"))

        # ---- constants ----
        ident = cp.tile([128, 128], F32, name="ident")
        make_identity(nc, ident)
        mb_sb = cp.tile([128, N_KT], F32, name="mb_sb")
        nc.sync.dma_start(mb_sb, mb)
        bq_sb = cp.tile([128, N_PAIRS], F32, name="bq_sb")
        nc.sync.dma_start(bq_sb, bq)
        bk_sb = cp.tile([128, N_PAIRS], F32, name="bk_sb")
        nc.sync.dma_start(bk_sb, bk)
        bv_sb = cp.tile([128, N_PAIRS * 130], F32, name="bv_sb")
        nc.sync.dma_start(bv_sb, bv)

        # ---- streamed loads as one 3D "mega" tile per tensor: each DMA
        # instruction covers one 512-column chunk across all 8 dt tiles, so
        # the Sync engine issues 15 input DMAs instead of ~120 (issue cost is
        # ~0.6us per DMA instruction on the queue-owning engine). ----
        def alloc_x(pfx):
            mega = xp.tile([128, N_DT, SQ], BF16, name=f"{pfx}m", tag="x")
            return mega, [mega[:, dt_i, :] for dt_i in range(N_DT)]

        def load_x_chunk(mega, xap, c):
            lo, hi = c * S_CHUNK, (c + 1) * S_CHUNK
            nc.sync.dma_start(
                mega[:, :, lo:hi], xap[:, :, lo:hi].rearrange("d p c -> p d c")
            )

        def alloc_w(pfx, width):
            mega = wp.tile([128, N_DT, width], BF16, name=f"{pfx}m", tag=f"{pfx}m")
            return mega, [mega[:, dt_i, :] for dt_i in range(N_DT)]

        def load_w_cols(mega, wap, lo, hi):
            nc.sync.dma_start(
                mega[:, :, lo:hi], wap[:, :, lo:hi].rearrange("d p c -> p d c")
            )

        xq_m, xq_sb = alloc_x("xq")
        xk_m, xk_sb = alloc_x("xk")
        xv_m, xv_sb = alloc_x("xv")
        wq_m, wq_sb = alloc_w("wq", HALF)
        wk_m, wk_sb = alloc_w("wk", HALF)
        wv_m, wv_sb = alloc_w("wv", N_PAIRS * 130)
        # arrival order == first-use order of the filler/attention streams;
        # weights are pair/group-sliced so the first scores wait on ~2.5MB
        load_w_cols(wq_m, wq, 0, 128)
        load_x_chunk(xq_m, xq, 0)
        load_w_cols(wk_m, wk, 0, 128)
        load_x_chunk(xk_m, xk, 0)
        load_x_chunk(xk_m, xk, 1)
        load_x_chunk(xk_m, xk, 2)
        load_x_chunk(xk_m, xk, 3)
        load_w_cols(wv_m, wv, 0, 260)
        load_x_chunk(xv_m, xv, 0)
        load_x_chunk(xq_m, xq, 1)
        load_w_cols(wq_m, wq, 128, HALF)
        load_w_cols(wk_m, wk, 128, HALF)
        load_x_chunk(xv_m, xv, 1)
        load_w_cols(wv_m, wv, 260, N_PAIRS * 130)
        load_x_chunk(xv_m, xv, 2)
        load_x_chunk(xq_m, xq, 2)
        load_x_chunk(xv_m, xv, 3)
        load_x_chunk(xq_m, xq, 3)

        v_tiles = {}  # (pair, kt) -> [128, 130] bf16 tile
        qkT = {}  # (pfx, pair) -> [128, SQ] bf16 tile
        qk_done = set()  # (pfx, pair, qc) fully emitted projection chunks

        def qk_tile(pfx, pair):
            if (pfx, pair) not in qkT:
                qkT[(pfx, pair)] = qkvp.tile(
                    [128, SQ], BF16, name=f"{pfx}T{pair}", tag=f"{pfx}T", bufs=2
                )
            return qkT[(pfx, pair)]

        proj_ps_open = {}

        def emit_qk_chunk(pair, pfx, qc, half=None):
            # one [128, 512] projection chunk: 8 accumulating MMs + bias copy.
            # half=0/1 emits only the first/second 4 contraction MMs (filler
            # granularity); half=None emits the whole chunk.
            dst = qk_tile(pfx, pair)
            w_sb = wq_sb if pfx == "q" else wk_sb
            b_sb = bq_sb if pfx == "q" else bk_sb
            x_sb = xq_sb if pfx == "q" else xk_sb
            key = (pair, pfx, qc)
            if half == 1:
                ps = proj_ps_open.pop(key)
            else:
                ps = proj_ps.tile(
                    [128, S_CHUNK], F32, name=f"{pfx}ps{pair}_{qc}", tag="proj"
                )
            dts = range(N_DT) if half is None else range(half * 4, half * 4 + 4)
            for dt_i in dts:
                nc.tensor.matmul(
                    ps,
                    lhsT=w_sb[dt_i][:, pair * 128 : (pair + 1) * 128],
                    rhs=x_sb[dt_i][:, qc * S_CHUNK : (qc + 1) * S_CHUNK],
                    start=(dt_i == 0),
                    stop=(dt_i == N_DT - 1),
                )
            if half == 0:
                proj_ps_open[key] = ps
            else:
                nc.vector.tensor_scalar_add(
                    dst[:, qc * S_CHUNK : (qc + 1) * S_CHUNK],
                    ps,
                    b_sb[:, pair : pair + 1],
                )
                qk_done.add((pfx, pair, qc))

        v_ps_open = {}

        def emit_v_chunk(g, st, half=None):
            # v projection for pairs (2g, 2g+1), one key tile: N=260 matmuls.
            # half=0/1 splits the 8 contraction MMs for filler granularity.
            if half == 1:
                ps = v_ps_open.pop((g, st))
            else:
                ps = proj_ps.tile([128, S_CHUNK], F32, name=f"vps{g}_{st}", tag="proj")
            dts = range(N_DT) if half is None else range(half * 4, half * 4 + 4)
            for dt_i in dts:
                nc.tensor.matmul(
                    ps[:, 0:260],
                    lhsT=xv_sb[dt_i][:, st * 128 : (st + 1) * 128],
                    rhs=wv_sb[dt_i][:, g * 260 : (g + 1) * 260],
                    start=(dt_i == 0),
                    stop=(dt_i == N_DT - 1),
                )
            if half == 0:
                v_ps_open[(g, st)] = ps
                return
            for j in range(2):
                pair = 2 * g + j
                vt = qkvp.tile(
                    [128, 130], BF16, name=f"v{pair}_{st}", tag="v", bufs=4 * N_KT
                )
                nc.vector.tensor_add(
                    vt,
                    ps[:, j * 130 : (j + 1) * 130],
                    bv_sb[:, pair * 130 : (pair + 1) * 130],
                )
                v_tiles[(pair, st)] = vt

        # filler queue: ALL deferred projection work in ~0.43us halves, popped
        # two per block (straddling the scores so the in-order PE queue never
        # stalls on the ACTIVATE ping-pong). Ordered by DMA arrival and by
        # need-by (python emission must precede readers).
        filler = []  # (cost_us, deadline_block, emit_fn)

        def _qk_half(pair, pfx, qc, half):
            return lambda: emit_qk_chunk(pair, pfx, qc, half)

        def _v_half(g, st, half):
            return lambda: emit_v_chunk(g, st, half)

        def add_qk(pair, pfx, qc, dl):
            filler.append((0.88, dl, _qk_half(pair, pfx, qc, 0)))
            filler.append((0.88, dl, _qk_half(pair, pfx, qc, 1)))

        def add_v(g, st, dl):
            filler.append((0.45, dl, _v_half(g, st, 0)))
            filler.append((0.45, dl, _v_half(g, st, 1)))

        # deadline = block whose emitted scores/AVs read the produced tile
        add_qk(0, "q", 1, 7)
        for st in range(0, 6):
            add_v(0, st, 8 + st)
        add_qk(0, "q", 2, 15)
        for st in range(6, N_KT):
            add_v(0, st, 8 + st)
        add_qk(0, "q", 3, 23)
        for c in range(N_QC):
            add_qk(1, "k", c, 31 + 2 * c)
        add_qk(1, "q", 0, 31)
        for st in range(N_KT):
            add_v(1, st, 36 + st)
        add_qk(1, "q", 1, 39)
        add_qk(1, "q", 2, 47)
        add_qk(1, "q", 3, 55)
        for c in range(N_QC):
            add_qk(2, "k", c, 52 + 2 * c)
        for c in range(N_QC):
            add_qk(2, "q", c, 50 + 6 * c)
        for c in range(N_QC):
            add_qk(3, "k", c, 74 + 2 * c)
        for c in range(N_QC):
            add_qk(3, "q", c, 78 + 5 * c)

        filler.sort(key=lambda e: e[1])
        filler_total = sum(e[0] for e in filler)
        spent = [0.0]

        def pop_filler(b):
            # forced pops: deadline due next block (correctness)
            popped = 0.0
            while filler and filler[0][1] <= b + 1:
                cost, _, fn = filler.pop(0)
                fn()
                spent[0] += cost
                popped += cost
            # uniform reserve: keep global pace so the tail stays fed
            target = filler_total * (b + 1) / 100.0
            while (
                filler
                and popped < 1.0
                and spent[0] + filler[0][0] <= target + 0.9
            ):
                cost, _, fn = filler.pop(0)
                fn()
                spent[0] += cost
                popped += cost

        # prologue: first-scores deps, plus the k chunks (their DMA chunks
        # arrive during the otherwise PE-idle startup window)
        emit_qk_chunk(0, "q", 0)
        emit_qk_chunk(0, "k", 0)
        for c in range(1, N_QC):
            emit_qk_chunk(0, "k", c)

        # ---- software-pipelined attention stream over (pair, qc, kt) ----
        iters = [
            (pair, qc, kt)
            for pair in range(N_PAIRS)
            for qc in range(N_QC)
            for kt in range(N_KT)
        ]
        sc_map = {}
        av_map = {}

        def emit_scores(i):
            pair, qc, kt = iters[i]
            assert ("q", pair, qc) in qk_done, f"q chunk not ready for iter {i}"
            assert ("k", pair, kt // 4) in qk_done, f"k chunk not ready for iter {i}"
            qT = qk_tile("q", pair)
            kT = qk_tile("k", pair)
            sc = sc_ps.tile([128, 1024], F32, name=f"sc{pair}_{qc}_{kt}", tag="sc")
            # scoresT for heads A and B, packed in PE row groups
            nc.tensor.matmul(
                sc[:, 0:512],
                lhsT=kT[0:64, kt * 128 : (kt + 1) * 128],
                rhs=qT[0:64, qc * S_CHUNK : (qc + 1) * S_CHUNK],
                start=True,
                stop=True,
            )
            nc.tensor.matmul(
                sc[:, 512:1024],
                lhsT=kT[64:128, kt * 128 : (kt + 1) * 128],
                rhs=qT[64:128, qc * S_CHUNK : (qc + 1) * S_CHUNK],
                start=True,
                stop=True,
            )
            sc_map[i] = sc

        def emit_epilogue(pair, qc, av_a, av_b):
            # transpose back to [q, d_v], normalize, store
            stg3 = stgp.tile([128, 4, 128], F32, name=f"st{pair}_{qc}", tag="stg")
            stgs = [stg3[:, u, :] for u in range(4)]
            for h_i, av in enumerate((av_a, av_b)):
                avt = avtp.tile(
                    [65, S_CHUNK], F32, name=f"avt{pair}_{qc}_{h_i}", tag="avt"
                )
                nc.vector.tensor_copy(avt, av)
                tp = tp_ps.tile([128, 260], F32, name=f"tp{pair}_{qc}_{h_i}", tag="tp")
                for u in range(4):
                    nc.tensor.transpose(
                        tp[:, u * 65 : u * 65 + 65],
                        avt[:, u * 128 : (u + 1) * 128],
                        ident[0:65, 0:65],
                    )
                rc = rp.tile([128, 4], F32, name=f"rc{pair}_{qc}_{h_i}", tag="rc")
                nc.vector.reciprocal(rc, tp[:, 64:260:65])
                for u in range(4):
                    nc.vector.tensor_scalar_mul(
                        stgs[u][:, h_i * 64 : (h_i + 1) * 64],
                        tp[:, u * 65 : u * 65 + 64],
                        rc[:, u : u + 1],
                    )
            nc.sync.dma_start(
                out[qc * 4 : (qc + 1) * 4, :, pair * 128 : (pair + 1) * 128]
                .rearrange("u p c -> p u c"),
                stg3,
            )

        def emit_av(pair, qc, kt, ex):
            if kt == 0:
                av_map[(pair, qc)] = (
                    av_ps.tile([65, S_CHUNK], F32, name=f"ava{pair}_{qc}", tag="av"),
                    av_ps.tile([65, S_CHUNK], F32, name=f"avb{pair}_{qc}", tag="av"),
                )
            av_a, av_b = av_map[(pair, qc)]
            nc.tensor.matmul(
                av_a,
                lhsT=v_tiles[(pair, kt)][:, 0:65],
                rhs=ex[:, 0:512],
                start=(kt == 0),
                stop=(kt == N_KT - 1),
            )
            nc.tensor.matmul(
                av_b,
                lhsT=v_tiles[(pair, kt)][:, 65:130],
                rhs=ex[:, 512:1024],
                start=(kt == 0),
                stop=(kt == N_KT - 1),
            )

        # Emission in 2-iteration blocks, software-pipelined:
        #   block b: exps (2b, 2b+1) | AV burst (2b-2, 2b-1) | scores (2b+2,
        #   2b+3) | one filler unit. The AV inputs are always two blocks old,
        #   so the 4-matmul AV burst never waits mid-stream; batching halves
        #   the PE stream-switch tax. Iters 0..15 (pair 0, qc 0) defer their
        #   AVs entirely so ScalarE starts while the v projection still waits
        #   on the xv DMA (~50us).
        emit_scores(0)
        emit_scores(1)
        ex_map = {}
        n_it = len(iters)

        def emit_av_i(i):
            pair, qc, kt = iters[i]
            emit_av(pair, qc, kt, ex_map.pop(i))
            if kt == N_KT - 1:
                emit_epilogue(pair, qc, *av_map.pop((pair, qc)))

        av_cursor = 0

        def av_ready(i):
            pair, qc, kt = iters[i]
            return (pair, kt) in v_tiles

        # Block b: exps (2b, 2b+1) | AV catch-up (lag-1 target, max 4, gated
        # on the v projection) | filler half | scores (2b+2, 2b+3) | filler
        # half. Fillers straddle the scores so the in-order PE queue reaches
        # the scores only after the previous ACTIVATE has freed their psum.
        for b in range(n_it // 2):
            i0, i1 = 2 * b, 2 * b + 1
            for i in (i0, i1):
                pair, qc, kt = iters[i]
                ex = expp.tile(
                    [128, 1024], BF16, name=f"ex{pair}_{qc}_{kt}", tag="ex", bufs=21
                )
                ex_map[i] = ex
                nc.scalar.activation(
                    ex,
                    sc_map.pop(i),
                    mybir.ActivationFunctionType.Exp,
                    bias=mb_sb[:, kt : kt + 1],
                    scale=0.125,
                )
            if b > 8:
                navs = 0
                while (
                    av_cursor <= min(2 * b - 1, n_it - 1)
                    and navs < 4
                    and av_ready(av_cursor)
                ):
                    emit_av_i(av_cursor)
                    av_cursor += 1
                    navs += 1
            pop_filler(b)
            if i1 + 2 < n_it:
                emit_scores(i1 + 1)
                emit_scores(i1 + 2)
        # drain remaining AVs + epilogues
        while filler:
            cost, _, fn = filler.pop(0)
            fn()
        while av_cursor < n_it:
            assert av_ready(av_cursor)
            emit_av_i(av_cursor)
            av_cursor += 1

        assert not filler, f"{len(filler)} filler chunks left unscheduled"
        assert not ex_map and not av_map and not sc_map

        assert not filler, f"{len(filler)} filler chunks left unscheduled"


def _prep_core_inputs(pre_qs, pre_ks, pre_vs, k_mask, q_w, q_b, k_w, k_b, v_w, v_b, core):
    b = core // 2
    hh = core % 2
    cols = slice(HALF * hh, HALF * (hh + 1))

    xq = np.ascontiguousarray(pre_qs[b].T).astype(BF16_NP).reshape(N_DT, 128, SQ)
    xk = np.ascontiguousarray(pre_ks[b].T).astype(BF16_NP).reshape(N_DT, 128, SK)
    xv = np.ascontiguousarray(pre_vs[b].T).astype(BF16_NP).reshape(N_DT, 128, SK)
    wq = np.ascontiguousarray(q_w[:, cols]).astype(BF16_NP).reshape(N_DT, 128, HALF)
    wk = np.ascontiguousarray(k_w[:, cols]).astype(BF16_NP).reshape(N_DT, 128, HALF)

    wv_core = v_w[:, cols].astype(np.float32)
    wv = np.zeros((D_PRE, N_PAIRS * 130), dtype=np.float32)
    bv_core = v_b[cols].astype(np.float32)
    bv_ext = np.zeros(N_PAIRS * 130, dtype=np.float32)
    for p in range(N_PAIRS):
        wv[:, p * 130 : p * 130 + 64] = wv_core[:, p * 128 : p * 128 + 64]
        wv[:, p * 130 + 65 : p * 130 + 129] = wv_core[:, p * 128 + 64 : p * 128 + 128]
        bv_ext[p * 130 : p * 130 + 64] = bv_core[p * 128 : p * 128 + 64]
        bv_ext[p * 130 + 64] = 1.0
        bv_ext[p * 130 + 65 : p * 130 + 129] = bv_core[p * 128 + 64 : p * 128 + 128]
        bv_ext[p * 130 + 129] = 1.0

    bq = np.ascontiguousarray(q_b[cols].astype(np.float32).reshape(N_PAIRS, 128).T)
    bk = np.ascontiguousarray(k_b[cols].astype(np.float32).reshape(N_PAIRS, 128).T)
    bv_full = np.ascontiguousarray(np.tile(bv_ext[None, :], (128, 1)))

    # mask True -> 0.0, False -> MASK_NEG
    mbias = np.where(k_mask[b], 0.0, MASK_NEG).astype(np.float32)
    mb = np.ascontiguousarray(mbias.reshape(N_KT, 128).T)

    return {
        "xq": xq,
        "xk": xk,
        "xv": xv,
        "wq": wq,
        "wk": wk,
        "wv": wv.astype(BF16_NP).reshape(N_DT, 128, N_PAIRS * 130),
        "bq": bq,
        "bk": bk,
        "bv": bv_full,
        "mb": mb,
    }


def kernel(pre_qs, pre_ks, pre_vs, k_mask, q_w, q_b, k_w, k_b, v_w, v_b):
    global _COMPILED
    args = (pre_qs, pre_ks, pre_vs, k_mask, q_w, q_b, k_w, k_b, v_w, v_b)
    args = tuple(np.asarray(a) for a in args)

    if _COMPILED is None:
        _COMPILED = _build_program()
    nc = _COMPILED

    in_maps = [_prep_core_inputs(*args, core=c) for c in range(N_CORES)]

    trace = bool(int(os.environ.get("BASS_KERNEL_TRACE", "0")))
    res = run_bass_kernel_spmd(
        nc,
        in_maps,
        core_ids=list(range(N_CORES)),
        trace=trace,
    )
    if trace:
        kernel.last_results = res

    out = np.empty((B, SQ, H * D_V), dtype=np.float32)
    for c in range(N_CORES):
        b = c // 2
        hh = c % 2
        out[b, :, HALF * hh : HALF * (hh + 1)] = res.results[c]["out"].reshape(SQ, HALF)
    return out

